# revision 7
# baseline (speedup 1.0000x reference)
"""AttentionalPooler Trainium2 kernel.

Full inputs -> full outputs; internally data-parallel over batch across 8
NeuronCores (b=8, one batch element per core).

Per-core math (one batch element, all in fp32):
  xk  = LN(x)                      [4096, 1024]
  q   = (LN(query) @ Wq) * scale   [256, 1024]   (identical on every core)
  kT  = Wk'^T @ xk^T               [1024, 4096]  (K stored transposed)
  V   = xk @ Wv'                   [4096, 1024]  (row-major, +ones col/head)
  S^T = kT_h^T-slices @ qT_h       [4096, 256] per head  (j on partitions)
  E   = exp(S^T)  (no max subtraction; |S| <= ~7 so fp32-safe)
  [O^T_h; den_h] = [V_h | 1]^T @ E  accumulated over j   [65, 256]
  out = sum_h (O_h / den_h) @ Wout_h                     [256, 1024]

LN gamma and the attention scale are folded into the weights host-side;
LN beta becomes a bias vector applied at PSUM evacuation.
"""

import os
import sys
import types

for _p in ("/root/.axon_site", "/root/.axon_site/_ro/trn_rl_repo", "/opt/trn_rl_repo"):
    if os.path.isdir(_p) and _p not in sys.path:
        sys.path.append(_p)

# The image's antenv package lacks axon_hooks; shim it with the ctypes-based
# NTFF hook from trn_agent_boot so trace=True works under axon.
try:
    import antenv.axon_hooks  # noqa: F401
except ImportError:
    try:
        import trn_agent_boot.trn_boot as _tb

        _hook = _tb._ntff_profile_via_ctypes("/opt/axon/libaxon_pjrt.so")
    except Exception:
        _hook = None
    _m = types.ModuleType("antenv.axon_hooks")
    _m.get_axon_ntff_profile_hook = lambda: _hook
    sys.modules["antenv.axon_hooks"] = _m

import numpy as np

import concourse.bass as bass
import concourse.tile as tile
from concourse import mybir
from concourse.masks import make_identity

D = 1024          # model dim == ctx dim
NCTX = 4096       # keys per batch element
NQ = 256          # queries
H = 16            # heads
DH = 64           # head dim
NCORES = 8
EPS = 1e-5
QTR = 1024        # keys processed per resident chunk (4 quarters)
SUP = 512         # kT-projection moving-dim tile

F32 = mybir.dt.float32


def _patch_drain(max_waits=1):
    """This walrus build rejects >1 sync-wait on the SP Drain that Tile emits
    at kernel exit. Split the waits across a chain of drains."""

    def patched(self, tick_clock, wait_clock):
        from concourse.vector_clock import ScopedClock

        drain_inst = self.nc.sync.drain()
        wait_clock.add_sem_waits(
            drain_inst.ins, ScopedClock({None: tick_clock.global_clock})
        )
        si = drain_inst.ins.sync_info
        waits = list(si.on_wait or []) if si else []
        if len(waits) > max_waits:
            si.on_wait = waits[:max_waits]
            rest = waits[max_waits:]
            while rest:
                extra = self.nc.sync.drain()
                extra.ins.sync_info = mybir.SyncInfo(
                    on_wait=rest[:max_waits], on_update=[]
                )
                rest = rest[max_waits:]
        self.nc.all_engine_barrier()
        assert self.sems is not None
        popped = self.nc._tile_sem_poison_stack.pop()
        assert popped is self._sem_poison
        self.nc.clear_and_free_semaphores(list(self.sems.allocated().values()))
        self.nc.all_engine_barrier()

    tile.TileContext._drain_and_barrier = patched


_patch_drain()


def _split_sync_waits(nc, max_waits=1):
    """This walrus build rejects instructions carrying more than one sync
    wait. Hoist excess waits onto same-engine NoOps placed just before the
    owning instruction (engine queues are serial, so this is equivalent)."""
    for f in nc.m.functions:
        for bb in f.blocks:
            new_list = []
            changed = False
            for inst in bb.instructions:
                si = inst.sync_info
                waits = list(si.on_wait) if si and si.on_wait else []
                if len(waits) > max_waits:
                    changed = True
                    keep = waits[-max_waits:]
                    rest = waits[:-max_waits]
                    k = 0
                    while rest:
                        carrier = mybir.InstNoOp(
                            name=f"{inst.name}-w{k}", ins=[], outs=[]
                        )
                        carrier.engine = inst.engine
                        carrier.sync_info = mybir.SyncInfo(
                            on_wait=rest[:max_waits], on_update=[]
                        )
                        rest = rest[max_waits:]
                        k += 1
                        nc.register_instruction(carrier, overwrite=True)
                        new_list.append(carrier)
                    si.on_wait = keep
                new_list.append(inst)
            if changed:
                bb.instructions = new_list


def _layernorm_rows(nc, pools, xt, p=128):
    """In-place LN (pure normalize) of xt [p, D] along the free dim."""
    per = pools["per"]
    stats = per.tile([p, 2, nc.vector.BN_STATS_DIM], F32, tag="stats")
    for sg in range(2):
        nc.vector.bn_stats(
            out=stats[:, sg, :], in_=xt[:, sg * 512:(sg + 1) * 512]
        )
    mv = per.tile([p, nc.vector.BN_AGGR_DIM], F32, tag="mv")
    nc.vector.bn_aggr(out=mv, in_=stats)
    rstd = per.tile([p, 1], F32, tag="rstd")
    nc.scalar.activation(
        out=rstd, in_=mv[:, 1:2], func=mybir.ActivationFunctionType.Sqrt,
        bias=pools["eps"], scale=1.0,
    )
    nc.vector.reciprocal(out=rstd, in_=rstd)
    nc.vector.tensor_scalar(
        out=xt, in0=xt, scalar1=mv[:, 0:1], scalar2=rstd,
        op0=mybir.AluOpType.subtract, op1=mybir.AluOpType.mult,
    )


def build_program():
    nc = bass.Bass("TRN2", target_bir_lowering=False, debug=False)

    x = nc.dram_tensor("x", [NCTX, D], F32, kind="ExternalInput").ap()
    qry = nc.dram_tensor("qry", [NQ, D], F32, kind="ExternalInput").ap()
    wq = nc.dram_tensor("wq", [D, D], F32, kind="ExternalInput").ap()
    wk = nc.dram_tensor("wk", [D, D], F32, kind="ExternalInput").ap()
    wv = nc.dram_tensor("wv", [D, D], F32, kind="ExternalInput").ap()
    wo = nc.dram_tensor("wo", [D, D], F32, kind="ExternalInput").ap()
    bq = nc.dram_tensor("bq", [128, 8], F32, kind="ExternalInput").ap()
    bk = nc.dram_tensor("bk", [128, 8], F32, kind="ExternalInput").ap()
    bv = nc.dram_tensor("bv", [D], F32, kind="ExternalInput").ap()
    out = nc.dram_tensor("out", [NQ, D], F32, kind="ExternalOutput").ap()

    with tile.TileContext(nc) as tc:
        _build_body(nc, tc, x, qry, wq, wk, wv, wo, bq, bk, bv, out)
    _split_sync_waits(nc)
    return nc


def _build_body(nc, tc, x, qry, wq, wk, wv, wo, bq, bk, bv, out):
    import contextlib

    ctx = contextlib.ExitStack()
    with ctx:
        consts = ctx.enter_context(tc.tile_pool(name="consts", bufs=1))
        wpool = ctx.enter_context(tc.tile_pool(name="wpool", bufs=1))
        wstream = ctx.enter_context(tc.tile_pool(name="wstream", bufs=2))
        xpool = ctx.enter_context(tc.tile_pool(name="xpool", bufs=2))
        big = ctx.enter_context(tc.tile_pool(name="big", bufs=1))
        per = ctx.enter_context(tc.tile_pool(name="per", bufs=3))
        etp = ctx.enter_context(tc.tile_pool(name="etp", bufs=3))
        outp = ctx.enter_context(tc.tile_pool(name="outp", bufs=2))
        ps_tr = ctx.enter_context(tc.tile_pool(name="ps_tr", bufs=1, space="PSUM"))
        ps_mm = ctx.enter_context(tc.tile_pool(name="ps_mm", bufs=2, space="PSUM"))
        ps_st = ctx.enter_context(tc.tile_pool(name="ps_st", bufs=2, space="PSUM"))
        ps_ot = ctx.enter_context(tc.tile_pool(name="ps_ot", bufs=2, space="PSUM"))

        pools = {"per": per}

        # constants
        ident = consts.tile([128, 128], F32, tag="ident")
        make_identity(nc, ident)
        eps_t = consts.tile([128, 1], F32, tag="eps")
        nc.vector.memset(eps_t, EPS)
        pools["eps"] = eps_t
        ones_t = consts.tile([128, 64], F32, tag="ones")
        nc.vector.memset(ones_t, 1.0)
        bq_sb = consts.tile([128, 8], F32, tag="bq")
        nc.sync.dma_start(out=bq_sb, in_=bq)
        bk_sb = consts.tile([128, 8], F32, tag="bk")
        nc.sync.dma_start(out=bk_sb, in_=bk)
        bv_rep = consts.tile([128, D], F32, tag="bvrep")
        bv_bcast = bass.AP(tensor=bv.tensor, offset=bv.offset,
                           ap=[[0, 128]] + list(bv.ap))
        nc.gpsimd.dma_start(out=bv_rep, in_=bv_bcast)

        # resident weights: wk_sb/wv_sb [128, dchunk, e]
        wk_r = wk.rearrange("(c p) e -> p c e", p=128)
        wv_r = wv.rearrange("(c p) e -> p c e", p=128)
        wk_sb = wpool.tile([128, 8, D], F32, tag="wk")
        nc.sync.dma_start(out=wk_sb, in_=wk_r)
        wv_sb = wpool.tile([128, 8, D], F32, tag="wv")
        nc.sync.dma_start(out=wv_sb, in_=wv_r)

        # ---- phase 0: q = LN(query) @ Wq' + bq, stored transposed ----
        # tag-shared with xkT: qnT is dead once qT is built
        qnT_full = big.tile([128, 8, SUP], F32, tag="xkT")
        qnT = qnT_full[:, :, :NQ]
        for t in range(2):
            qt = xpool.tile([128, D], F32, tag="xt")
            nc.sync.dma_start(out=qt, in_=qry[t * 128:(t + 1) * 128, :])
            _layernorm_rows(nc, pools, qt)
            ptr = ps_tr.tile([128, 8, 128], F32, tag="tr")
            for dc in range(8):
                nc.tensor.transpose(
                    ptr[:, dc, :], qt[:, dc * 128:(dc + 1) * 128], ident
                )
            nc.vector.tensor_copy(out=qnT[:, :, t * 128:(t + 1) * 128], in_=ptr)

        qT = consts.tile([128, 8, NQ], F32, tag="qT")  # [e', echunk, i]
        wq_r = wq.rearrange("(c p) e -> p c e", p=128)
        for ec in range(8):
            wq_t = wstream.tile([128, 8, 128], F32, tag="wqs")
            nc.sync.dma_start(out=wq_t, in_=wq_r[:, :, ec * 128:(ec + 1) * 128])
            psq = ps_mm.tile([128, NQ], F32, tag="mm")
            for dc in range(8):
                nc.tensor.matmul(
                    psq, lhsT=wq_t[:, dc, :], rhs=qnT[:, dc, :],
                    start=(dc == 0), stop=(dc == 7),
                )
            nc.scalar.activation(
                out=qT[:, ec, :], in_=psq,
                func=mybir.ActivationFunctionType.Identity,
                bias=bq_sb[:, ec:ec + 1], scale=1.0,
            )

        # accumulators: [O^T_h ; den_h] per head, accumulated over quarters
        otacc = big.tile([65, H, NQ], F32, tag="ot")

        nq_qtr = NCTX // QTR
        for qtr in range(nq_qtr):
            # ---- A: kT and V' for this quarter ----
            kT_q = big.tile([128, 8, QTR], F32, tag="kt")   # [e', echunk, j]
            v_q = big.tile([128, QTR // 128, H * 65], F32, tag="vq")
            for s in range(QTR // SUP):
                xkT = big.tile([128, 8, SUP], F32, tag="xkT")  # [d', dchunk, j]
                for jt in range(SUP // 128):
                    j0 = qtr * QTR + s * SUP + jt * 128
                    xt = xpool.tile([128, D], F32, tag="xt")
                    nc.sync.dma_start(out=xt, in_=x[j0:j0 + 128, :])
                    _layernorm_rows(nc, pools, xt)
                    ptr = ps_tr.tile([128, 8, 128], F32, tag="tr")
                    for dc in range(8):
                        nc.tensor.transpose(
                            ptr[:, dc, :], xt[:, dc * 128:(dc + 1) * 128], ident
                        )
                    nc.vector.tensor_copy(
                        out=xkT[:, :, jt * 128:(jt + 1) * 128], in_=ptr
                    )
                # kT += Wk'^T @ xk^T for the 512 new j columns
                for ec in range(8):
                    psk = ps_mm.tile([128, SUP], F32, tag="mm")
                    for dc in range(8):
                        nc.tensor.matmul(
                            psk,
                            lhsT=wk_sb[:, dc, ec * 128:(ec + 1) * 128],
                            rhs=xkT[:, dc, :],
                            start=(dc == 0), stop=(dc == 7),
                        )
                    nc.scalar.activation(
                        out=kT_q[:, ec, s * SUP:(s + 1) * SUP], in_=psk,
                        func=mybir.ActivationFunctionType.Identity,
                        bias=bk_sb[:, ec:ec + 1], scale=1.0,
                    )
                # V rows for the 512 new j, interleaved 64 cols + ones per head
                for jt in range(SUP // 128):
                    jj = s * (SUP // 128) + jt
                    for nt in range(2):
                        psv = ps_mm.tile([128, SUP], F32, tag="mm")
                        for dc in range(8):
                            nc.tensor.matmul(
                                psv,
                                lhsT=xkT[:, dc, jt * 128:(jt + 1) * 128],
                                rhs=wv_sb[:, dc, nt * 512:(nt + 1) * 512],
                                start=(dc == 0), stop=(dc == 7),
                            )
                        vdst = v_q[:, jj, nt * 8 * 65:(nt + 1) * 8 * 65].rearrange(
                            "p (h c) -> p h c", c=65
                        )[:, :, 0:64]
                        nc.vector.tensor_add(
                            out=vdst,
                            in0=psv.rearrange("p (h c) -> p h c", c=64),
                            in1=bv_rep[:, nt * 512:(nt + 1) * 512].rearrange(
                                "p (h c) -> p h c", c=64
                            ),
                        )
            ones_view = v_q.rearrange("p j (h c) -> p j h c", c=65)[:, :, :, 64:65]
            nc.vector.memset(ones_view, 1.0)

            # ---- B: attention over this quarter's keys ----
            for h in range(16):
                pb = (h % 2) * 64
                pso = ps_ot.tile([65, NQ], F32, tag="ot")
                for jj in range(QTR // 128):
                    pst = ps_st.tile([128, NQ], F32, tag="st")
                    nc.tensor.matmul(
                        pst,
                        lhsT=kT_q[pb:pb + 64, h // 2, jj * 128:(jj + 1) * 128],
                        rhs=qT[pb:pb + 64, h // 2, :],
                        start=True, stop=True,
                    )
                    et = etp.tile([128, NQ], F32, tag="et")
                    nc.scalar.activation(
                        out=et, in_=pst, func=mybir.ActivationFunctionType.Exp
                    )
                    nc.tensor.matmul(
                        pso,
                        lhsT=v_q[:, jj, h * 65:(h + 1) * 65],
                        rhs=et,
                        start=(jj == 0), stop=(jj == QTR // 128 - 1),
                    )
                if qtr == 0:
                    nc.vector.tensor_copy(out=otacc[:, h, :], in_=pso)
                else:
                    nc.vector.tensor_add(
                        out=otacc[:, h, :], in0=otacc[:, h, :], in1=pso
                    )

        # ---- normalize: O_h /= den_h (den row kept in otacc row 64) ----
        nc.vector.reciprocal(out=otacc[64:65, :, :], in_=otacc[64:65, :, :])
        for h in range(16):
            psb = ps_st.tile([64, NQ], F32, tag="st")
            nc.tensor.matmul(
                psb, lhsT=ones_t[64:65, :], rhs=otacc[64:65, h, :],
                start=True, stop=True,
            )
            nc.vector.tensor_mul(
                out=otacc[0:64, h, :], in0=otacc[0:64, h, :], in1=psb
            )

        # ---- out = sum_h O_h @ Wout_h ----
        for ic in range(2):
            psf0 = ps_mm.tile([128, 512], F32, tag="mm")
            psf1 = ps_mm.tile([128, 512], F32, tag="mm")
            psf = [psf0, psf1]
            for h in range(16):
                wo_t = wstream.tile([64, D], F32, tag="wqs")
                nc.sync.dma_start(out=wo_t, in_=wo[h * 64:(h + 1) * 64, :])
                for ft in range(2):
                    nc.tensor.matmul(
                        psf[ft],
                        lhsT=otacc[0:64, h, ic * 128:(ic + 1) * 128],
                        rhs=wo_t[:, ft * 512:(ft + 1) * 512],
                        start=(h == 0), stop=(h == 15),
                    )
            ot = outp.tile([128, D], F32, tag="outsb")
            for ft in range(2):
                nc.scalar.activation(
                    out=ot[:, ft * 512:(ft + 1) * 512], in_=psf[ft],
                    func=mybir.ActivationFunctionType.Copy,
                )
            nc.sync.dma_start(out=out[ic * 128:(ic + 1) * 128, :], in_=ot)


_CACHED = None


def _get_program():
    global _CACHED
    if _CACHED is None:
        _CACHED = build_program()
    return _CACHED


def _prep_inputs(x, query, Wq, Wkv, Wout, ln_q_g, ln_q_b, ln_k_g, ln_k_b):
    scale = DH ** -0.5
    f32 = np.float32
    Wq = np.asarray(Wq, f32)
    Wkv = np.asarray(Wkv, f32)
    Wout = np.asarray(Wout, f32)
    wq_eff = (np.asarray(ln_q_g, f32)[:, None] * Wq * scale).astype(f32)
    bq_eff = (np.asarray(ln_q_b, f32) @ Wq * scale).astype(f32)
    wk_eff = (np.asarray(ln_k_g, f32)[:, None] * Wkv[:, :D]).astype(f32)
    bk_eff = (np.asarray(ln_k_b, f32) @ Wkv[:, :D]).astype(f32)
    wv_eff = (np.asarray(ln_k_g, f32)[:, None] * Wkv[:, D:]).astype(f32)
    bv_eff = (np.asarray(ln_k_b, f32) @ Wkv[:, D:]).astype(f32)
    shared = {
        "qry": np.ascontiguousarray(np.asarray(query, f32)),
        "wq": np.ascontiguousarray(wq_eff),
        "wk": np.ascontiguousarray(wk_eff),
        "wv": np.ascontiguousarray(wv_eff),
        "wo": np.ascontiguousarray(Wout),
        "bq": np.ascontiguousarray(bq_eff.reshape(8, 128).T),
        "bk": np.ascontiguousarray(bk_eff.reshape(8, 128).T),
        "bv": np.ascontiguousarray(bv_eff),
    }
    x = np.asarray(x, f32)
    in_maps = [
        dict(shared, x=np.ascontiguousarray(x[i])) for i in range(NCORES)
    ]
    return in_maps


def run(trace=False, **inputs):
    from concourse.bass_utils import run_bass_kernel_spmd

    nc = _get_program()
    in_maps = _prep_inputs(**inputs)
    res = run_bass_kernel_spmd(
        nc, in_maps, core_ids=list(range(NCORES)), trace=trace
    )
    out = np.stack([res.results[i]["out"] for i in range(NCORES)], axis=0)
    return out.astype(np.float32), res.exec_time_ns


def kernel(**inputs):
    out, _ = run(trace=False, **inputs)
    return out


# revision 8
# speedup vs baseline: 2.2639x; 2.2639x over previous
"""AttentionalPooler Trainium2 kernel.

Full inputs -> full outputs; internally data-parallel over batch across 8
NeuronCores (b=8, one batch element per core).

Per-core math (one batch element, all in fp32):
  xk  = LN(x)                      [4096, 1024]
  q   = (LN(query) @ Wq) * scale   [256, 1024]   (identical on every core)
  kT  = Wk'^T @ xk^T               [1024, 4096]  (K stored transposed)
  V   = xk @ Wv'                   [4096, 1024]  (row-major, +ones col/head)
  S^T = kT_h^T-slices @ qT_h       [4096, 256] per head  (j on partitions)
  E   = exp(S^T)  (no max subtraction; |S| <= ~7 so fp32-safe)
  [O^T_h; den_h] = [V_h | 1]^T @ E  accumulated over j   [65, 256]
  out = sum_h (O_h / den_h) @ Wout_h                     [256, 1024]

LN gamma and the attention scale are folded into the weights host-side;
LN beta becomes a bias vector applied at PSUM evacuation.
"""

import os
import sys
import types

for _p in ("/root/.axon_site", "/root/.axon_site/_ro/trn_rl_repo", "/opt/trn_rl_repo"):
    if os.path.isdir(_p) and _p not in sys.path:
        sys.path.append(_p)

# The image's antenv package lacks axon_hooks; shim it with the ctypes-based
# NTFF hook from trn_agent_boot so trace=True works under axon.
try:
    import antenv.axon_hooks  # noqa: F401
except ImportError:
    try:
        import trn_agent_boot.trn_boot as _tb

        _hook = _tb._ntff_profile_via_ctypes("/opt/axon/libaxon_pjrt.so")
    except Exception:
        _hook = None
    _m = types.ModuleType("antenv.axon_hooks")
    _m.get_axon_ntff_profile_hook = lambda: _hook
    sys.modules["antenv.axon_hooks"] = _m

import numpy as np

import concourse.bass as bass
import concourse.tile as tile
from concourse import mybir
from concourse.masks import make_identity

D = 1024          # model dim == ctx dim
NCTX = 4096       # keys per batch element
NQ = 256          # queries
H = 16            # heads
DH = 64           # head dim
NCORES = 8
EPS = 1e-5
QTR = 1024        # keys processed per resident chunk (4 quarters)
SUP = 512         # kT-projection moving-dim tile

F32 = mybir.dt.float32
BF16 = mybir.dt.bfloat16

# dtype for matmul operands (PSUM always accumulates fp32; LN, exp and
# softmax normalization always run in fp32)
MM_DT = BF16


def _mm_np():
    if MM_DT == F32:
        return np.float32
    import ml_dtypes

    return ml_dtypes.bfloat16


def _patch_drain(max_waits=1):
    """This walrus build rejects >1 sync-wait on the SP Drain that Tile emits
    at kernel exit. Split the waits across a chain of drains."""

    def patched(self, tick_clock, wait_clock):
        from concourse.vector_clock import ScopedClock

        drain_inst = self.nc.sync.drain()
        wait_clock.add_sem_waits(
            drain_inst.ins, ScopedClock({None: tick_clock.global_clock})
        )
        si = drain_inst.ins.sync_info
        waits = list(si.on_wait or []) if si else []
        if len(waits) > max_waits:
            si.on_wait = waits[:max_waits]
            rest = waits[max_waits:]
            while rest:
                extra = self.nc.sync.drain()
                extra.ins.sync_info = mybir.SyncInfo(
                    on_wait=rest[:max_waits], on_update=[]
                )
                rest = rest[max_waits:]
        self.nc.all_engine_barrier()
        assert self.sems is not None
        popped = self.nc._tile_sem_poison_stack.pop()
        assert popped is self._sem_poison
        self.nc.clear_and_free_semaphores(list(self.sems.allocated().values()))
        self.nc.all_engine_barrier()

    tile.TileContext._drain_and_barrier = patched


_patch_drain()


def _split_sync_waits(nc, max_waits=1):
    """This walrus build rejects instructions carrying more than one sync
    wait. Hoist excess waits onto same-engine NoOps placed just before the
    owning instruction (engine queues are serial, so this is equivalent)."""
    for f in nc.m.functions:
        for bb in f.blocks:
            new_list = []
            changed = False
            for inst in bb.instructions:
                si = inst.sync_info
                waits = list(si.on_wait) if si and si.on_wait else []
                if len(waits) > max_waits:
                    changed = True
                    keep = waits[-max_waits:]
                    rest = waits[:-max_waits]
                    k = 0
                    while rest:
                        carrier = mybir.InstNoOp(
                            name=f"{inst.name}-w{k}", ins=[], outs=[]
                        )
                        carrier.engine = inst.engine
                        carrier.sync_info = mybir.SyncInfo(
                            on_wait=rest[:max_waits], on_update=[]
                        )
                        rest = rest[max_waits:]
                        k += 1
                        nc.register_instruction(carrier, overwrite=True)
                        new_list.append(carrier)
                    si.on_wait = keep
                new_list.append(inst)
            if changed:
                bb.instructions = new_list


def _layernorm_rows(nc, pools, xt, p=128):
    """In-place LN (pure normalize) of xt [p, D] along the free dim."""
    per = pools["per"]
    stats = per.tile([p, 2, nc.vector.BN_STATS_DIM], F32, tag="stats")
    for sg in range(2):
        nc.vector.bn_stats(
            out=stats[:, sg, :], in_=xt[:, sg * 512:(sg + 1) * 512]
        )
    mv = per.tile([p, nc.vector.BN_AGGR_DIM], F32, tag="mv")
    nc.vector.bn_aggr(out=mv, in_=stats)
    rstd = per.tile([p, 1], F32, tag="rstd")
    nc.scalar.activation(
        out=rstd, in_=mv[:, 1:2], func=mybir.ActivationFunctionType.Sqrt,
        bias=pools["eps"], scale=1.0,
    )
    nc.vector.reciprocal(out=rstd, in_=rstd)
    nc.vector.tensor_scalar(
        out=xt, in0=xt, scalar1=mv[:, 0:1], scalar2=rstd,
        op0=mybir.AluOpType.subtract, op1=mybir.AluOpType.mult,
    )


def build_program():
    nc = bass.Bass("TRN2", target_bir_lowering=False, debug=False)

    x = nc.dram_tensor("x", [NCTX, D], F32, kind="ExternalInput").ap()
    qry = nc.dram_tensor("qry", [NQ, D], F32, kind="ExternalInput").ap()
    wq = nc.dram_tensor("wq", [D, D], MM_DT, kind="ExternalInput").ap()
    wk = nc.dram_tensor("wk", [D, D], MM_DT, kind="ExternalInput").ap()
    wv = nc.dram_tensor("wv", [D, D], MM_DT, kind="ExternalInput").ap()
    wo = nc.dram_tensor("wo", [D, D], MM_DT, kind="ExternalInput").ap()
    bq = nc.dram_tensor("bq", [128, 8], F32, kind="ExternalInput").ap()
    bk = nc.dram_tensor("bk", [128, 8], F32, kind="ExternalInput").ap()
    bv = nc.dram_tensor("bv", [D], F32, kind="ExternalInput").ap()
    out = nc.dram_tensor("out", [NQ, D], F32, kind="ExternalOutput").ap()

    with tile.TileContext(nc) as tc:
        _build_body(nc, tc, x, qry, wq, wk, wv, wo, bq, bk, bv, out)
    _split_sync_waits(nc)
    return nc


def _build_body(nc, tc, x, qry, wq, wk, wv, wo, bq, bk, bv, out):
    import contextlib

    ctx = contextlib.ExitStack()
    with ctx:
        consts = ctx.enter_context(tc.tile_pool(name="consts", bufs=1))
        wpool = ctx.enter_context(tc.tile_pool(name="wpool", bufs=1))
        wstream = ctx.enter_context(tc.tile_pool(name="wstream", bufs=2))
        xpool = ctx.enter_context(tc.tile_pool(name="xpool", bufs=2))
        big = ctx.enter_context(tc.tile_pool(name="big", bufs=1))
        per = ctx.enter_context(tc.tile_pool(name="per", bufs=3))
        etp = ctx.enter_context(tc.tile_pool(name="etp", bufs=3))
        outp = ctx.enter_context(tc.tile_pool(name="outp", bufs=2))
        ps_tr = ctx.enter_context(tc.tile_pool(name="ps_tr", bufs=1, space="PSUM"))
        ps_mm = ctx.enter_context(tc.tile_pool(name="ps_mm", bufs=2, space="PSUM"))
        ps_st = ctx.enter_context(tc.tile_pool(name="ps_st", bufs=2, space="PSUM"))
        ps_ot = ctx.enter_context(tc.tile_pool(name="ps_ot", bufs=2, space="PSUM"))

        pools = {"per": per}

        # constants
        ident = consts.tile([128, 128], F32, tag="ident")
        make_identity(nc, ident)
        eps_t = consts.tile([128, 1], F32, tag="eps")
        nc.vector.memset(eps_t, EPS)
        pools["eps"] = eps_t
        ones_t = consts.tile([128, 64], F32, tag="ones")
        nc.vector.memset(ones_t, 1.0)
        bq_sb = consts.tile([128, 8], F32, tag="bq")
        nc.sync.dma_start(out=bq_sb, in_=bq)
        bk_sb = consts.tile([128, 8], F32, tag="bk")
        nc.sync.dma_start(out=bk_sb, in_=bk)
        bv_rep = consts.tile([128, D], F32, tag="bvrep")
        bv_bcast = bass.AP(tensor=bv.tensor, offset=bv.offset,
                           ap=[[0, 128]] + list(bv.ap))
        nc.gpsimd.dma_start(out=bv_rep, in_=bv_bcast)

        # resident weights: wk_sb/wv_sb [128, dchunk, e]
        wk_r = wk.rearrange("(c p) e -> p c e", p=128)
        wv_r = wv.rearrange("(c p) e -> p c e", p=128)
        wk_sb = wpool.tile([128, 8, D], MM_DT, tag="wk")
        nc.sync.dma_start(out=wk_sb, in_=wk_r)
        wv_sb = wpool.tile([128, 8, D], MM_DT, tag="wv")
        nc.sync.dma_start(out=wv_sb, in_=wv_r)

        # ---- phase 0: q = LN(query) @ Wq' + bq, stored transposed ----
        # tag-shared with xkT: qnT is dead once qT is built
        qnT_full = big.tile([128, 8, SUP], MM_DT, tag="xkT")
        qnT = qnT_full[:, :, :NQ]
        for t in range(2):
            qt = xpool.tile([128, D], F32, tag="xt")
            nc.sync.dma_start(out=qt, in_=qry[t * 128:(t + 1) * 128, :])
            _layernorm_rows(nc, pools, qt)
            ptr = ps_tr.tile([128, 8, 128], F32, tag="tr")
            for dc in range(8):
                nc.tensor.transpose(
                    ptr[:, dc, :], qt[:, dc * 128:(dc + 1) * 128], ident
                )
            nc.vector.tensor_copy(out=qnT[:, :, t * 128:(t + 1) * 128], in_=ptr)

        qT = consts.tile([128, 8, NQ], MM_DT, tag="qT")  # [e', echunk, i]
        wq_r = wq.rearrange("(c p) e -> p c e", p=128)
        for ec in range(8):
            wq_t = wstream.tile([128, 8, 128], MM_DT, tag="wqs")
            nc.sync.dma_start(out=wq_t, in_=wq_r[:, :, ec * 128:(ec + 1) * 128])
            psq = ps_mm.tile([128, NQ], F32, tag="mm")
            for dc in range(8):
                nc.tensor.matmul(
                    psq, lhsT=wq_t[:, dc, :], rhs=qnT[:, dc, :],
                    start=(dc == 0), stop=(dc == 7),
                )
            nc.scalar.activation(
                out=qT[:, ec, :], in_=psq,
                func=mybir.ActivationFunctionType.Identity,
                bias=bq_sb[:, ec:ec + 1], scale=1.0,
            )

        # accumulators: [O^T_h ; den_h] per head, accumulated over quarters
        otacc = big.tile([65, H, NQ], F32, tag="ot")

        nq_qtr = NCTX // QTR
        for qtr in range(nq_qtr):
            # ---- A: kT and V' for this quarter ----
            kT_q = big.tile([128, 8, QTR], MM_DT, tag="kt")   # [e', echunk, j]
            v_q = big.tile([128, QTR // 128, H * 65], MM_DT, tag="vq")
            for s in range(QTR // SUP):
                xkT = big.tile([128, 8, SUP], MM_DT, tag="xkT")  # [d', dchunk, j]
                for jt in range(SUP // 128):
                    j0 = qtr * QTR + s * SUP + jt * 128
                    xt = xpool.tile([128, D], F32, tag="xt")
                    nc.sync.dma_start(out=xt, in_=x[j0:j0 + 128, :])
                    _layernorm_rows(nc, pools, xt)
                    ptr = ps_tr.tile([128, 8, 128], F32, tag="tr")
                    for dc in range(8):
                        nc.tensor.transpose(
                            ptr[:, dc, :], xt[:, dc * 128:(dc + 1) * 128], ident
                        )
                    nc.vector.tensor_copy(
                        out=xkT[:, :, jt * 128:(jt + 1) * 128], in_=ptr
                    )
                # kT += Wk'^T @ xk^T for the 512 new j columns
                for ec in range(8):
                    psk = ps_mm.tile([128, SUP], F32, tag="mm")
                    for dc in range(8):
                        nc.tensor.matmul(
                            psk,
                            lhsT=wk_sb[:, dc, ec * 128:(ec + 1) * 128],
                            rhs=xkT[:, dc, :],
                            start=(dc == 0), stop=(dc == 7),
                        )
                    nc.scalar.activation(
                        out=kT_q[:, ec, s * SUP:(s + 1) * SUP], in_=psk,
                        func=mybir.ActivationFunctionType.Identity,
                        bias=bk_sb[:, ec:ec + 1], scale=1.0,
                    )
                # V rows for the 512 new j, interleaved 64 cols + ones per head
                for jt in range(SUP // 128):
                    jj = s * (SUP // 128) + jt
                    for nt in range(2):
                        psv = ps_mm.tile([128, SUP], F32, tag="mm")
                        for dc in range(8):
                            nc.tensor.matmul(
                                psv,
                                lhsT=xkT[:, dc, jt * 128:(jt + 1) * 128],
                                rhs=wv_sb[:, dc, nt * 512:(nt + 1) * 512],
                                start=(dc == 0), stop=(dc == 7),
                            )
                        vdst = v_q[:, jj, nt * 8 * 65:(nt + 1) * 8 * 65].rearrange(
                            "p (h c) -> p h c", c=65
                        )[:, :, 0:64]
                        nc.vector.tensor_add(
                            out=vdst,
                            in0=psv.rearrange("p (h c) -> p h c", c=64),
                            in1=bv_rep[:, nt * 512:(nt + 1) * 512].rearrange(
                                "p (h c) -> p h c", c=64
                            ),
                        )
            ones_view = v_q.rearrange("p j (h c) -> p j h c", c=65)[:, :, :, 64:65]
            nc.vector.memset(ones_view, 1.0)

            # ---- B: attention over this quarter's keys ----
            for h in range(16):
                pb = (h % 2) * 64
                pso = ps_ot.tile([65, NQ], F32, tag="ot")
                for jj in range(QTR // 128):
                    pst = ps_st.tile([128, NQ], F32, tag="st")
                    nc.tensor.matmul(
                        pst,
                        lhsT=kT_q[pb:pb + 64, h // 2, jj * 128:(jj + 1) * 128],
                        rhs=qT[pb:pb + 64, h // 2, :],
                        start=True, stop=True,
                    )
                    et = etp.tile([128, NQ], MM_DT, tag="et")
                    nc.scalar.activation(
                        out=et, in_=pst, func=mybir.ActivationFunctionType.Exp
                    )
                    nc.tensor.matmul(
                        pso,
                        lhsT=v_q[:, jj, h * 65:(h + 1) * 65],
                        rhs=et,
                        start=(jj == 0), stop=(jj == QTR // 128 - 1),
                    )
                if qtr == 0:
                    nc.vector.tensor_copy(out=otacc[:, h, :], in_=pso)
                else:
                    nc.vector.tensor_add(
                        out=otacc[:, h, :], in0=otacc[:, h, :], in1=pso
                    )

        # ---- normalize: O_h /= den_h (den row kept in otacc row 64) ----
        ot_n = big.tile([64, H, NQ], MM_DT, tag="otn")
        nc.vector.reciprocal(out=otacc[64:65, :, :], in_=otacc[64:65, :, :])
        for h in range(16):
            psb = ps_st.tile([64, NQ], F32, tag="st")
            nc.tensor.matmul(
                psb, lhsT=ones_t[64:65, :], rhs=otacc[64:65, h, :],
                start=True, stop=True,
            )
            nc.vector.tensor_mul(
                out=ot_n[:, h, :], in0=otacc[0:64, h, :], in1=psb
            )

        # ---- out = sum_h O_h @ Wout_h ----
        for ic in range(2):
            psf0 = ps_mm.tile([128, 512], F32, tag="mm")
            psf1 = ps_mm.tile([128, 512], F32, tag="mm")
            psf = [psf0, psf1]
            for h in range(16):
                wo_t = wstream.tile([64, D], MM_DT, tag="wos")
                nc.sync.dma_start(out=wo_t, in_=wo[h * 64:(h + 1) * 64, :])
                for ft in range(2):
                    nc.tensor.matmul(
                        psf[ft],
                        lhsT=ot_n[:, h, ic * 128:(ic + 1) * 128],
                        rhs=wo_t[:, ft * 512:(ft + 1) * 512],
                        start=(h == 0), stop=(h == 15),
                    )
            ot = outp.tile([128, D], F32, tag="outsb")
            for ft in range(2):
                nc.scalar.activation(
                    out=ot[:, ft * 512:(ft + 1) * 512], in_=psf[ft],
                    func=mybir.ActivationFunctionType.Copy,
                )
            nc.sync.dma_start(out=out[ic * 128:(ic + 1) * 128, :], in_=ot)


_CACHED = None


def _get_program():
    global _CACHED
    if _CACHED is None:
        _CACHED = build_program()
    return _CACHED


def _prep_inputs(x, query, Wq, Wkv, Wout, ln_q_g, ln_q_b, ln_k_g, ln_k_b):
    scale = DH ** -0.5
    f32 = np.float32
    Wq = np.asarray(Wq, f32)
    Wkv = np.asarray(Wkv, f32)
    Wout = np.asarray(Wout, f32)
    wq_eff = (np.asarray(ln_q_g, f32)[:, None] * Wq * scale).astype(f32)
    bq_eff = (np.asarray(ln_q_b, f32) @ Wq * scale).astype(f32)
    wk_eff = (np.asarray(ln_k_g, f32)[:, None] * Wkv[:, :D]).astype(f32)
    bk_eff = (np.asarray(ln_k_b, f32) @ Wkv[:, :D]).astype(f32)
    wv_eff = (np.asarray(ln_k_g, f32)[:, None] * Wkv[:, D:]).astype(f32)
    bv_eff = (np.asarray(ln_k_b, f32) @ Wkv[:, D:]).astype(f32)
    mdt = _mm_np()
    shared = {
        "qry": np.ascontiguousarray(np.asarray(query, f32)),
        "wq": np.ascontiguousarray(wq_eff.astype(mdt)),
        "wk": np.ascontiguousarray(wk_eff.astype(mdt)),
        "wv": np.ascontiguousarray(wv_eff.astype(mdt)),
        "wo": np.ascontiguousarray(Wout.astype(mdt)),
        "bq": np.ascontiguousarray(bq_eff.reshape(8, 128).T),
        "bk": np.ascontiguousarray(bk_eff.reshape(8, 128).T),
        "bv": np.ascontiguousarray(bv_eff),
    }
    x = np.asarray(x, f32)
    in_maps = [
        dict(shared, x=np.ascontiguousarray(x[i])) for i in range(NCORES)
    ]
    return in_maps


def run(trace=False, **inputs):
    from concourse.bass_utils import run_bass_kernel_spmd

    nc = _get_program()
    in_maps = _prep_inputs(**inputs)
    res = run_bass_kernel_spmd(
        nc, in_maps, core_ids=list(range(NCORES)), trace=trace
    )
    out = np.stack([res.results[i]["out"] for i in range(NCORES)], axis=0)
    return out.astype(np.float32), res.exec_time_ns


def kernel(**inputs):
    out, _ = run(trace=False, **inputs)
    return out


# revision 10
# speedup vs baseline: 2.3402x; 1.0337x over previous
"""AttentionalPooler Trainium2 kernel.

Full inputs -> full outputs; internally data-parallel over batch across 8
NeuronCores (b=8, one batch element per core).

Per-core math (one batch element, all in fp32):
  xk  = LN(x)                      [4096, 1024]
  q   = (LN(query) @ Wq) * scale   [256, 1024]   (identical on every core)
  kT  = Wk'^T @ xk^T               [1024, 4096]  (K stored transposed)
  V   = xk @ Wv'                   [4096, 1024]  (row-major, +ones col/head)
  S^T = kT_h^T-slices @ qT_h       [4096, 256] per head  (j on partitions)
  E   = exp(S^T)  (no max subtraction; |S| <= ~7 so fp32-safe)
  [O^T_h; den_h] = [V_h | 1]^T @ E  accumulated over j   [65, 256]
  out = sum_h (O_h / den_h) @ Wout_h                     [256, 1024]

LN gamma and the attention scale are folded into the weights host-side;
LN beta becomes a bias vector applied at PSUM evacuation.
"""

import os
import sys
import types

for _p in ("/root/.axon_site", "/root/.axon_site/_ro/trn_rl_repo", "/opt/trn_rl_repo"):
    if os.path.isdir(_p) and _p not in sys.path:
        sys.path.append(_p)

# The image's antenv package lacks axon_hooks; shim it with the ctypes-based
# NTFF hook from trn_agent_boot so trace=True works under axon.
try:
    import antenv.axon_hooks  # noqa: F401
except ImportError:
    try:
        import trn_agent_boot.trn_boot as _tb

        _hook = _tb._ntff_profile_via_ctypes("/opt/axon/libaxon_pjrt.so")
    except Exception:
        _hook = None
    _m = types.ModuleType("antenv.axon_hooks")
    _m.get_axon_ntff_profile_hook = lambda: _hook
    sys.modules["antenv.axon_hooks"] = _m

import numpy as np

import concourse.bass as bass
import concourse.tile as tile
from concourse import mybir
from concourse.masks import make_identity

D = 1024          # model dim == ctx dim
NCTX = 4096       # keys per batch element
NQ = 256          # queries
H = 16            # heads
DH = 64           # head dim
NCORES = 8
EPS = 1e-5
QTR = 1024        # keys processed per resident chunk (4 quarters)
SUP = 512         # kT-projection moving-dim tile

F32 = mybir.dt.float32
BF16 = mybir.dt.bfloat16

# dtype for matmul operands (PSUM always accumulates fp32; LN, exp and
# softmax normalization always run in fp32)
MM_DT = BF16


def _mm_np():
    if MM_DT == F32:
        return np.float32
    import ml_dtypes

    return ml_dtypes.bfloat16


def _patch_drain(max_waits=1):
    """This walrus build rejects >1 sync-wait on the SP Drain that Tile emits
    at kernel exit. Split the waits across a chain of drains."""

    def patched(self, tick_clock, wait_clock):
        from concourse.vector_clock import ScopedClock

        drain_inst = self.nc.sync.drain()
        wait_clock.add_sem_waits(
            drain_inst.ins, ScopedClock({None: tick_clock.global_clock})
        )
        si = drain_inst.ins.sync_info
        waits = list(si.on_wait or []) if si else []
        if len(waits) > max_waits:
            si.on_wait = waits[:max_waits]
            rest = waits[max_waits:]
            while rest:
                extra = self.nc.sync.drain()
                extra.ins.sync_info = mybir.SyncInfo(
                    on_wait=rest[:max_waits], on_update=[]
                )
                rest = rest[max_waits:]
        self.nc.all_engine_barrier()
        assert self.sems is not None
        popped = self.nc._tile_sem_poison_stack.pop()
        assert popped is self._sem_poison
        self.nc.clear_and_free_semaphores(list(self.sems.allocated().values()))
        self.nc.all_engine_barrier()

    tile.TileContext._drain_and_barrier = patched


_patch_drain()


def _split_sync_waits(nc, max_waits=1):
    """This walrus build rejects instructions carrying more than one sync
    wait. Hoist excess waits onto same-engine NoOps placed just before the
    owning instruction (engine queues are serial, so this is equivalent)."""
    for f in nc.m.functions:
        for bb in f.blocks:
            new_list = []
            changed = False
            for inst in bb.instructions:
                si = inst.sync_info
                waits = list(si.on_wait) if si and si.on_wait else []
                if len(waits) > max_waits:
                    changed = True
                    keep = waits[-max_waits:]
                    rest = waits[:-max_waits]
                    k = 0
                    while rest:
                        carrier = mybir.InstNoOp(
                            name=f"{inst.name}-w{k}", ins=[], outs=[]
                        )
                        carrier.engine = inst.engine
                        carrier.sync_info = mybir.SyncInfo(
                            on_wait=rest[:max_waits], on_update=[]
                        )
                        rest = rest[max_waits:]
                        k += 1
                        nc.register_instruction(carrier, overwrite=True)
                        new_list.append(carrier)
                    si.on_wait = keep
                new_list.append(inst)
            if changed:
                bb.instructions = new_list


def _layernorm_rows(nc, pools, xt, p=128):
    """In-place LN (pure normalize) of xt [p, D] along the free dim."""
    per = pools["per"]
    stats = per.tile([p, 2, nc.vector.BN_STATS_DIM], F32, tag="stats")
    for sg in range(2):
        nc.vector.bn_stats(
            out=stats[:, sg, :], in_=xt[:, sg * 512:(sg + 1) * 512]
        )
    mv = per.tile([p, nc.vector.BN_AGGR_DIM], F32, tag="mv")
    nc.vector.bn_aggr(out=mv, in_=stats)
    rstd = per.tile([p, 1], F32, tag="rstd")
    nc.scalar.activation(
        out=rstd, in_=mv[:, 1:2], func=mybir.ActivationFunctionType.Sqrt,
        bias=pools["eps"], scale=1.0,
    )
    nc.vector.reciprocal(out=rstd, in_=rstd)
    nc.vector.tensor_scalar(
        out=xt, in0=xt, scalar1=mv[:, 0:1], scalar2=rstd,
        op0=mybir.AluOpType.subtract, op1=mybir.AluOpType.mult,
    )


def build_program():
    nc = bass.Bass("TRN2", target_bir_lowering=False, debug=False)

    x = nc.dram_tensor("x", [NCTX, D], F32, kind="ExternalInput").ap()
    qry = nc.dram_tensor("qry", [NQ, D], F32, kind="ExternalInput").ap()
    wq = nc.dram_tensor("wq", [D, D], MM_DT, kind="ExternalInput").ap()
    wk = nc.dram_tensor("wk", [D, D], MM_DT, kind="ExternalInput").ap()
    wv = nc.dram_tensor("wv", [D, D], MM_DT, kind="ExternalInput").ap()
    wo = nc.dram_tensor("wo", [D, D], MM_DT, kind="ExternalInput").ap()
    bq = nc.dram_tensor("bq", [128, 8], F32, kind="ExternalInput").ap()
    bk = nc.dram_tensor("bk", [128, 8], F32, kind="ExternalInput").ap()
    bv = nc.dram_tensor("bv", [D], F32, kind="ExternalInput").ap()
    out = nc.dram_tensor("out", [NQ, D], F32, kind="ExternalOutput").ap()

    with tile.TileContext(nc) as tc:
        _build_body(nc, tc, x, qry, wq, wk, wv, wo, bq, bk, bv, out)
    _split_sync_waits(nc)
    return nc


def _build_body(nc, tc, x, qry, wq, wk, wv, wo, bq, bk, bv, out):
    import contextlib

    ctx = contextlib.ExitStack()
    with ctx:
        consts = ctx.enter_context(tc.tile_pool(name="consts", bufs=1))
        wpool = ctx.enter_context(tc.tile_pool(name="wpool", bufs=1))
        wstream = ctx.enter_context(tc.tile_pool(name="wstream", bufs=2))
        xpool = ctx.enter_context(tc.tile_pool(name="xpool", bufs=2))
        big = ctx.enter_context(tc.tile_pool(name="big", bufs=1))
        per = ctx.enter_context(tc.tile_pool(name="per", bufs=3))
        etp = ctx.enter_context(tc.tile_pool(name="etp", bufs=3))
        outp = ctx.enter_context(tc.tile_pool(name="outp", bufs=2))
        ps_mm = ctx.enter_context(tc.tile_pool(name="ps_mm", bufs=2, space="PSUM"))
        ps_st = ctx.enter_context(tc.tile_pool(name="ps_st", bufs=4, space="PSUM"))
        ps_ot = ctx.enter_context(tc.tile_pool(name="ps_ot", bufs=2, space="PSUM"))

        pools = {"per": per}

        # constants
        ident = consts.tile([128, 128], F32, tag="ident")
        make_identity(nc, ident)
        eps_t = consts.tile([128, 1], F32, tag="eps")
        nc.vector.memset(eps_t, EPS)
        pools["eps"] = eps_t
        ones_t = consts.tile([128, 64], F32, tag="ones")
        nc.vector.memset(ones_t, 1.0)
        bq_sb = consts.tile([128, 8], F32, tag="bq")
        nc.sync.dma_start(out=bq_sb, in_=bq)
        bk_sb = consts.tile([128, 8], F32, tag="bk")
        nc.sync.dma_start(out=bk_sb, in_=bk)
        bv_rep = consts.tile([128, D], F32, tag="bvrep")
        bv_bcast = bass.AP(tensor=bv.tensor, offset=bv.offset,
                           ap=[[0, 128]] + list(bv.ap))
        nc.gpsimd.dma_start(out=bv_rep, in_=bv_bcast)

        # resident weights: wk_sb/wv_sb [128, dchunk, e]
        wk_r = wk.rearrange("(c p) e -> p c e", p=128)
        wv_r = wv.rearrange("(c p) e -> p c e", p=128)
        wk_sb = wpool.tile([128, 8, D], MM_DT, tag="wk")
        nc.sync.dma_start(out=wk_sb, in_=wk_r)
        wv_sb = wpool.tile([128, 8, D], MM_DT, tag="wv")
        nc.sync.dma_start(out=wv_sb, in_=wv_r)

        # ---- phase 0: q = LN(query) @ Wq' + bq, stored transposed ----
        # tag-shared with xkT: qnT is dead once qT is built
        qnT_full = big.tile([128, 8, SUP], MM_DT, tag="xkT")
        qnT = qnT_full[:, :, :NQ]
        for t in range(2):
            qt = xpool.tile([128, D], F32, tag="xt")
            nc.sync.dma_start(out=qt, in_=qry[t * 128:(t + 1) * 128, :])
            _layernorm_rows(nc, pools, qt)
            for c in range(4):
                ptr = ps_st.tile([128, 2, 128], F32, tag="st")
                for k in range(2):
                    dc = c * 2 + k
                    nc.tensor.transpose(
                        ptr[:, k, :], qt[:, dc * 128:(dc + 1) * 128], ident
                    )
                nc.vector.tensor_copy(
                    out=qnT[:, c * 2:c * 2 + 2, t * 128:(t + 1) * 128], in_=ptr
                )

        qT = consts.tile([128, 8, NQ], MM_DT, tag="qT")  # [e', echunk, i]
        wq_r = wq.rearrange("(c p) e -> p c e", p=128)
        for ec in range(8):
            wq_t = wstream.tile([128, 8, 128], MM_DT, tag="wqs")
            nc.sync.dma_start(out=wq_t, in_=wq_r[:, :, ec * 128:(ec + 1) * 128])
            psq = ps_mm.tile([128, NQ], F32, tag="mm")
            for dc in range(8):
                nc.tensor.matmul(
                    psq, lhsT=wq_t[:, dc, :], rhs=qnT[:, dc, :],
                    start=(dc == 0), stop=(dc == 7),
                )
            nc.vector.tensor_scalar(
                out=qT[:, ec, :], in0=psq, scalar1=bq_sb[:, ec:ec + 1],
                scalar2=None, op0=mybir.AluOpType.add,
            )

        # accumulators: [O^T_h ; den_h] per head, accumulated over quarters
        otacc = big.tile([65, H, NQ], F32, tag="ot")

        nq_qtr = NCTX // QTR
        for qtr in range(nq_qtr):
            # ---- A: kT and V' for this quarter ----
            kT_q = big.tile([128, 8, QTR], MM_DT, tag="kt")   # [e', echunk, j]
            v_q = big.tile([128, QTR // 128, H * 65], MM_DT, tag="vq")
            for s in range(QTR // SUP):
                xkT = big.tile([128, 8, SUP], MM_DT, tag="xkT")  # [d', dchunk, j]
                for jt in range(SUP // 128):
                    j0 = qtr * QTR + s * SUP + jt * 128
                    xt = xpool.tile([128, D], F32, tag="xt")
                    nc.sync.dma_start(out=xt, in_=x[j0:j0 + 128, :])
                    _layernorm_rows(nc, pools, xt)
                    for c in range(4):
                        ptr = ps_st.tile([128, 2, 128], F32, tag="st")
                        for k in range(2):
                            dc = c * 2 + k
                            nc.tensor.transpose(
                                ptr[:, k, :], xt[:, dc * 128:(dc + 1) * 128], ident
                            )
                        nc.vector.tensor_copy(
                            out=xkT[:, c * 2:c * 2 + 2, jt * 128:(jt + 1) * 128],
                            in_=ptr,
                        )
                # kT += Wk'^T @ xk^T for the 512 new j columns
                for ec in range(8):
                    psk = ps_mm.tile([128, SUP], F32, tag="mm")
                    for dc in range(8):
                        nc.tensor.matmul(
                            psk,
                            lhsT=wk_sb[:, dc, ec * 128:(ec + 1) * 128],
                            rhs=xkT[:, dc, :],
                            start=(dc == 0), stop=(dc == 7),
                        )
                    nc.vector.tensor_scalar(
                        out=kT_q[:, ec, s * SUP:(s + 1) * SUP], in0=psk,
                        scalar1=bk_sb[:, ec:ec + 1], scalar2=None,
                        op0=mybir.AluOpType.add,
                    )
                # V rows for the 512 new j, interleaved 64 cols + ones per head
                for jt in range(SUP // 128):
                    jj = s * (SUP // 128) + jt
                    for nt in range(2):
                        psv = ps_mm.tile([128, SUP], F32, tag="mm")
                        for dc in range(8):
                            nc.tensor.matmul(
                                psv,
                                lhsT=xkT[:, dc, jt * 128:(jt + 1) * 128],
                                rhs=wv_sb[:, dc, nt * 512:(nt + 1) * 512],
                                start=(dc == 0), stop=(dc == 7),
                            )
                        vdst = v_q[:, jj, nt * 8 * 65:(nt + 1) * 8 * 65].rearrange(
                            "p (h c) -> p h c", c=65
                        )[:, :, 0:64]
                        nc.vector.tensor_add(
                            out=vdst,
                            in0=psv.rearrange("p (h c) -> p h c", c=64),
                            in1=bv_rep[:, nt * 512:(nt + 1) * 512].rearrange(
                                "p (h c) -> p h c", c=64
                            ),
                        )
            ones_view = v_q.rearrange("p j (h c) -> p j h c", c=65)[:, :, :, 64:65]
            nc.vector.memset(ones_view, 1.0)

            # ---- B: attention, two heads of a chunk interleaved ----
            for hc in range(8):
                pso0 = ps_ot.tile([65, NQ], F32, tag="ot")
                pso1 = ps_ot.tile([65, NQ], F32, tag="ot")
                psos = (pso0, pso1)
                njj = QTR // 128
                for jj in range(njj):
                    for par in range(2):
                        h = hc * 2 + par
                        pb = par * 64
                        pst = ps_st.tile([128, NQ], F32, tag="st")
                        nc.tensor.matmul(
                            pst,
                            lhsT=kT_q[pb:pb + 64, hc, jj * 128:(jj + 1) * 128],
                            rhs=qT[pb:pb + 64, hc, :],
                            start=True, stop=True,
                        )
                        et = etp.tile([128, NQ], MM_DT, tag="et")
                        nc.scalar.activation(
                            out=et, in_=pst,
                            func=mybir.ActivationFunctionType.Exp,
                        )
                        nc.tensor.matmul(
                            psos[par],
                            lhsT=v_q[:, jj, h * 65:(h + 1) * 65],
                            rhs=et,
                            start=(jj == 0), stop=(jj == njj - 1),
                        )
                for par in range(2):
                    h = hc * 2 + par
                    if qtr == 0:
                        nc.vector.tensor_copy(out=otacc[:, h, :], in_=psos[par])
                    else:
                        nc.vector.tensor_add(
                            out=otacc[:, h, :], in0=otacc[:, h, :], in1=psos[par]
                        )

        # ---- normalize: O_h /= den_h (den row kept in otacc row 64) ----
        ot_n = big.tile([64, H, NQ], MM_DT, tag="otn")
        nc.vector.reciprocal(out=otacc[64:65, :, :], in_=otacc[64:65, :, :])
        for h in range(16):
            psb = ps_st.tile([64, NQ], F32, tag="st")
            nc.tensor.matmul(
                psb, lhsT=ones_t[64:65, :], rhs=otacc[64:65, h, :],
                start=True, stop=True,
            )
            nc.vector.tensor_mul(
                out=ot_n[:, h, :], in0=otacc[0:64, h, :], in1=psb
            )

        # ---- out = sum_h O_h @ Wout_h ----
        for ic in range(2):
            psf0 = ps_mm.tile([128, 512], F32, tag="mm")
            psf1 = ps_mm.tile([128, 512], F32, tag="mm")
            psf = [psf0, psf1]
            for h in range(16):
                wo_t = wstream.tile([64, D], MM_DT, tag="wos")
                nc.sync.dma_start(out=wo_t, in_=wo[h * 64:(h + 1) * 64, :])
                for ft in range(2):
                    nc.tensor.matmul(
                        psf[ft],
                        lhsT=ot_n[:, h, ic * 128:(ic + 1) * 128],
                        rhs=wo_t[:, ft * 512:(ft + 1) * 512],
                        start=(h == 0), stop=(h == 15),
                    )
            ot = outp.tile([128, D], F32, tag="outsb")
            for ft in range(2):
                nc.scalar.activation(
                    out=ot[:, ft * 512:(ft + 1) * 512], in_=psf[ft],
                    func=mybir.ActivationFunctionType.Copy,
                )
            nc.sync.dma_start(out=out[ic * 128:(ic + 1) * 128, :], in_=ot)


_CACHED = None


def _get_program():
    global _CACHED
    if _CACHED is None:
        _CACHED = build_program()
    return _CACHED


def _prep_inputs(x, query, Wq, Wkv, Wout, ln_q_g, ln_q_b, ln_k_g, ln_k_b):
    scale = DH ** -0.5
    f32 = np.float32
    Wq = np.asarray(Wq, f32)
    Wkv = np.asarray(Wkv, f32)
    Wout = np.asarray(Wout, f32)
    wq_eff = (np.asarray(ln_q_g, f32)[:, None] * Wq * scale).astype(f32)
    bq_eff = (np.asarray(ln_q_b, f32) @ Wq * scale).astype(f32)
    wk_eff = (np.asarray(ln_k_g, f32)[:, None] * Wkv[:, :D]).astype(f32)
    bk_eff = (np.asarray(ln_k_b, f32) @ Wkv[:, :D]).astype(f32)
    wv_eff = (np.asarray(ln_k_g, f32)[:, None] * Wkv[:, D:]).astype(f32)
    bv_eff = (np.asarray(ln_k_b, f32) @ Wkv[:, D:]).astype(f32)
    mdt = _mm_np()
    shared = {
        "qry": np.ascontiguousarray(np.asarray(query, f32)),
        "wq": np.ascontiguousarray(wq_eff.astype(mdt)),
        "wk": np.ascontiguousarray(wk_eff.astype(mdt)),
        "wv": np.ascontiguousarray(wv_eff.astype(mdt)),
        "wo": np.ascontiguousarray(Wout.astype(mdt)),
        "bq": np.ascontiguousarray(bq_eff.reshape(8, 128).T),
        "bk": np.ascontiguousarray(bk_eff.reshape(8, 128).T),
        "bv": np.ascontiguousarray(bv_eff),
    }
    x = np.asarray(x, f32)
    in_maps = [
        dict(shared, x=np.ascontiguousarray(x[i])) for i in range(NCORES)
    ]
    return in_maps


def run(trace=False, **inputs):
    from concourse.bass_utils import run_bass_kernel_spmd

    nc = _get_program()
    in_maps = _prep_inputs(**inputs)
    res = run_bass_kernel_spmd(
        nc, in_maps, core_ids=list(range(NCORES)), trace=trace
    )
    out = np.stack([res.results[i]["out"] for i in range(NCORES)], axis=0)
    return out.astype(np.float32), res.exec_time_ns


def kernel(**inputs):
    out, _ = run(trace=False, **inputs)
    return out
